# revision 26
# baseline (speedup 1.0000x reference)
"""GQA (B=1, S=2048, D=4096, H=32, G=8) on 8 TRN2 NeuronCores.

Sharding: tensor-parallel over heads - core c owns query heads 4c..4c+3 and
KV group c. Per core: bf16 qT/kT/vT projections from full x (transposed
layouts), RoPE, causal attention with transposed probs (exp without
max-subtraction - scores are bounded; denominator via a ones-column in V'),
normalization folded into a per-partition scalar multiply, per-chunk bf16
AllGather of ctxT, then a column-sharded Wo matmul producing a transposed
output tile.

v2 (software-pipelined): chunk sc+1's projections (and, on the last chunk,
phase-2 Wo blocks) are interleaved as PE filler inside chunk sc's attention
so the PE never waits on the Scalar engine's exp evictions; Scalar engine
runs exp only (rope rotation via DVE partition-offset muls with the rotation
sign pre-folded into the sin table; evictions on DVE; stores on gpsimd/
sync); chunk 0 runs all six projection chains kt-interleaved so the
DMA-paced start is PE-bound; phase-2 post-loop is kt-outer/ob-inner on the
idle scores PSUM banks so moving tiles release early; the last chunk's
AllGather is split in two (head-pairs) so the tail gather lands before
phase-2 needs it; startup DMAs in consumption order with chunk-major xT
host layout.  (XBAR DMA-transpose variants compute garbage on this HW -
see USE_XBAR_T - so V/ctx transposes stay on the PE.)

Self-contained: no sibling imports; hardcoded shapes.
"""
import contextlib
import ctypes
import os
import sys
import types

import ml_dtypes
import numpy as np

os.environ.setdefault("MYCRO_LOCAL_CACHE", "1")

for _p in ("/opt/trn_rl_repo", "/root/.axon_site/_ro/trn_rl_repo"):
    if _p not in sys.path and os.path.isdir(_p):
        sys.path.append(_p)

import concourse.bass as bass
import concourse.tile as tile
from concourse import mybir
from concourse.bass_utils import run_bass_kernel_spmd
from concourse.masks import make_identity

# ---------------------------------------------------------------- profiling shim
_SO_PATH = "/opt/axon/libaxon_pjrt.so"
_hook_holder = [None]


def _ntff_profile_via_ctypes(so_path):
    try:
        lib = ctypes.CDLL(so_path)
    except OSError:
        return None
    if not hasattr(lib, "axon_start_nrt_profile"):
        return None
    lib.axon_start_nrt_profile.argtypes = [
        ctypes.POINTER(ctypes.c_int64),
        ctypes.c_size_t,
    ]
    lib.axon_start_nrt_profile.restype = ctypes.c_int64
    lib.axon_stop_nrt_profile.argtypes = [ctypes.c_char_p]
    lib.axon_stop_nrt_profile.restype = ctypes.c_int64

    @contextlib.contextmanager
    def _hook(output_dir, device_ids):
        import jax

        jax.devices()
        if device_ids:
            ids = (ctypes.c_int64 * len(device_ids))(*device_ids)
            rc = lib.axon_start_nrt_profile(ids, len(device_ids))
        else:
            rc = lib.axon_start_nrt_profile(None, 0)
        if rc != 0:
            raise RuntimeError(f"axon_start_nrt_profile rc={rc}")
        try:
            yield
        finally:
            n = lib.axon_stop_nrt_profile(str(output_dir).encode())
            if n <= 0:
                print(f"WARNING: ntff capture wrote {n} files", file=sys.stderr)

    return _hook


def _install_prof_shim():
    if "antenv.axon_hooks" not in sys.modules:
        mod = types.ModuleType("antenv.axon_hooks")
        mod.set_axon_ntff_profile_hook = lambda h: _hook_holder.__setitem__(0, h)
        mod.get_axon_ntff_profile_hook = lambda: _hook_holder[0]
        sys.modules["antenv.axon_hooks"] = mod
    _hook_holder[0] = _ntff_profile_via_ctypes(_SO_PATH)
    import concourse.bass_utils as bu

    bu.upload_artifacts = lambda tmpdir: tmpdir


_install_prof_shim()

# ------------------------------------------------------------- wait-split pass
def _split_multi_waits(nc, maxw=1):
    """walrus in this container allows only one sync-wait per instruction;
    split extras onto nops inserted before the offender (same engine/block)."""

    def _remove_by_name(name):
        for f in nc.m.functions:
            for bb in f.blocks:
                for i, inst in enumerate(bb.instructions):
                    if inst.name == name:
                        lst = bb.instructions
                        del lst[i]
                        bb.instructions = lst
                        return inst
        raise KeyError(name)

    offenders = []
    for f in nc.m.functions:
        for bb in f.blocks:
            for inst in bb.instructions:
                si = inst.sync_info
                if si and si.on_wait and len(si.on_wait) > maxw:
                    offenders.append(inst.name)
    for name in offenders:
        target = None
        for f in nc.m.functions:
            for bb in f.blocks:
                for idx, inst in enumerate(bb.instructions):
                    if inst.name == name:
                        target = (bb, inst)
                        break
                if target:
                    break
            if target:
                break
        bb, inst = target
        waits = list(inst.sync_info.on_wait)
        updates = list(inst.sync_info.on_update or [])
        chunks = [waits[i:i + maxw] for i in range(0, len(waits), maxw)]
        nops = []
        for ch in chunks[:-1]:
            bnop = nc.engines[inst.engine].nop(nofuse=True, hint="waitsplit")
            nop_inst = _remove_by_name(bnop.ins.name)
            nop_inst.sync_info = mybir.SyncInfo(on_wait=ch, on_update=[])
            nops.append(nop_inst)
        inst.sync_info = mybir.SyncInfo(on_wait=chunks[-1], on_update=updates)
        lst = bb.instructions
        idx = next(i for i, x in enumerate(lst) if x.name == name)
        lst[idx:idx] = nops
        bb.instructions = lst
    return len(offenders)


# ------------------------------------------------------------------- constants
B, S, D = 1, 2048, 4096
H, G = 32, 8
HD = D // H            # 128
NC = 8                 # cores
HPC = H // NC          # heads per core = 4
OC = D // NC           # out columns per core = 512
P = 128
KT = D // P            # 32 contraction tiles
SCH = 512              # sequence chunk width
NSC = S // SCH         # 4
NQ = SCH // P          # 4 query subtiles per chunk
NKB = S // P           # 16 key tiles
SCALE = float(1.0 / np.sqrt(np.float32(HD)))

f32 = mybir.dt.float32
bf16 = mybir.dt.bfloat16

Copy = mybir.ActivationFunctionType.Copy
Exp = mybir.ActivationFunctionType.Exp

# HW-bisect flags (sim passes with both True)
USE_DVE_ROT = os.environ.get("K_DVE_ROT", "1") == "1"   # rope rot on DVE w/ partition offset
USE_XBAR_T = os.environ.get("K_XBAR_T", "0") == "1"     # XBAR DMA transpose: NaNs on HW (sim-only OK)


def _build_program():
    nc = bass.Bass()
    xT = nc.declare_dram_parameter("xT", [P, NSC, KT, SCH], bf16, isOutput=False)
    wq = nc.declare_dram_parameter("wq", [P, KT, OC], bf16, isOutput=False)
    wk = nc.declare_dram_parameter("wk", [P, KT, HD], bf16, isOutput=False)
    wv = nc.declare_dram_parameter("wv", [P, KT, HD], bf16, isOutput=False)
    wo = nc.declare_dram_parameter("wo", [P, KT, OC], bf16, isOutput=False)
    cosT = nc.declare_dram_parameter("cosT", [HD, S], bf16, isOutput=False)
    sinNT = nc.declare_dram_parameter("sinNT", [HD, S], bf16, isOutput=False)
    tri = nc.declare_dram_parameter("tri", [P, P], bf16, isOutput=False)
    outT = nc.declare_dram_parameter("outT", [OC, S], f32, isOutput=True)

    cc_ins = [nc.dram_tensor(f"cc_in{k}", [HPC * HD, SCH], bf16)
              for k in range(NSC)]
    cc_outs = [nc.dram_tensor(f"cc_out{k}", [D, SCH], bf16, addr_space="Shared")
               for k in range(NSC - 1)]
    # last chunk: two half-gathers (head-slots 01 / 23) so the tail gather
    # ends earlier; one collective's fixed cost is ~15us, so only 2-way
    cc3_outs = [nc.dram_tensor(f"cc3_out{j}", [NC * 2 * HD, SCH], bf16,
                               addr_space="Shared") for j in range(2)]

    with tile.TileContext(nc) as tc:
        with (
            tc.tile_pool(name="singles", bufs=1) as singles,
            tc.tile_pool(name="stream", bufs=14) as stream,
            tc.tile_pool(name="qts", bufs=8) as qtsp,
            tc.tile_pool(name="pt", bufs=18) as ptp,
            tc.tile_pool(name="work", bufs=6) as work,
            tc.tile_pool(name="ps", bufs=1, space="PSUM") as psp,
        ):
            # ---- startup DMAs, in first-consumption order: the K projection
            # needs wk + the first x tiles; then rope tables (K rope), wv,
            # then wq (Q projections).  wo is deferred to chunk 1.
            trim = singles.tile([P, P], bf16)
            nc.sync.dma_start(out=trim[:], in_=tri[:])
            wk_sb = singles.tile([P, KT, HD], bf16)
            wv_sb = singles.tile([P, KT, HD], bf16)
            wq_sb = singles.tile([P, KT, OC], bf16)
            wo_sb = singles.tile([P, KT, OC], bf16)   # DMA issued at chunk 1
            cos_sb = singles.tile([HD, S], bf16)
            sinN_sb = singles.tile([HD, S], bf16)

            # chunk-0 is DMA-paced: issue everything in kt-consumption order,
            # in ~1.25MB rounds of (wk, wv, wq, x) 4-kt slices so all six
            # chunk-0 projection chains can proceed as data lands.
            xtg0 = []
            for g in range(KT // 4):
                k4 = slice(4 * g, 4 * g + 4)
                t = stream.tile([P, 4, SCH], bf16, tag="stream")
                nc.sync.dma_start(out=wk_sb[:, k4, :], in_=wk[:, k4, :])
                if g < 2:
                    # round 0: the first K matmul's deps (wk, x kt0) must be
                    # the first triggers; weights follow the x halves
                    nc.sync.dma_start(out=t[:, 0:2, :],
                                      in_=xT[:, 0, 4 * g:4 * g + 2, :])
                    nc.sync.dma_start(out=wv_sb[:, k4, :], in_=wv[:, k4, :])
                    nc.sync.dma_start(out=wq_sb[:, k4, :], in_=wq[:, k4, :])
                    nc.sync.dma_start(out=t[:, 2:4, :],
                                      in_=xT[:, 0, 4 * g + 2:4 * g + 4, :])
                else:
                    nc.sync.dma_start(out=wv_sb[:, k4, :], in_=wv[:, k4, :])
                    nc.sync.dma_start(out=wq_sb[:, k4, :], in_=wq[:, k4, :])
                    nc.sync.dma_start(out=t[:], in_=xT[:, 0, k4, :])
                xtg0.append(t)
                if g == 3:
                    nc.sync.dma_start(out=cos_sb[:], in_=cosT[:])
                    nc.sync.dma_start(out=sinN_sb[:], in_=sinNT[:])

            # per-chunk K^T and V tiles (separate tiles so pipelined writes
            # to chunk sc+1 never alias reads of chunks <= sc)
            identb = None
            if not USE_XBAR_T:
                identb = singles.tile([P, P], bf16)
                make_identity(nc, identb[:])

            kTc = [singles.tile([HD, SCH], bf16, name=f"kTc{k}")
                   for k in range(NSC)]
            vpc = [singles.tile([P, NQ, HD + 1], bf16, name=f"vpc{k}")
                   for k in range(NSC)]
            for k in range(NSC):
                nc.gpsimd.memset(vpc[k][:, :, HD:HD + 1], 1.0)

            def rope_evict(ps_t, dst, dst0, tab0):
                """ps_t: PSUM [HD, SCH] pre-rope; writes dst[:, dst0:dst0+SCH]
                (bf16).  rot(t)=[-x2, x1] folded into the sign of sinNT rows."""
                m1 = work.tile([HD, SCH], f32, tag="m1", bufs=2)
                nc.vector.tensor_mul(m1[:], ps_t[:], cos_sb[:, tab0:tab0 + SCH])
                rot = work.tile([HD, SCH], f32, tag="rot", bufs=2)
                if USE_DVE_ROT:
                    nc.vector.tensor_mul(rot[0:64, :], ps_t[64:128, :],
                                         sinN_sb[0:64, tab0:tab0 + SCH])
                    nc.vector.tensor_mul(rot[64:128, :], ps_t[0:64, :],
                                         sinN_sb[64:128, tab0:tab0 + SCH])
                else:
                    nc.scalar.activation(out=rot[0:64, :], in_=ps_t[64:128, :],
                                         func=Copy)
                    nc.scalar.activation(out=rot[64:128, :], in_=ps_t[0:64, :],
                                         func=Copy)
                    nc.vector.tensor_mul(rot[:], rot[:],
                                         sinN_sb[:, tab0:tab0 + SCH])
                nc.vector.tensor_add(dst[:, dst0:dst0 + SCH], m1[:], rot[:])

            # ---------------- projection emission (one PE matmul per yield)
            def proj_gen(sc, xtg, qts_out, first=False):
                s0 = sc * SCH

                def xts(kt):
                    return xtg[kt // 4][:, kt % 4, :]

                def q_chain(h):
                    ps_q = psp.tile([P, SCH], f32, tag="p", bufs=2)
                    for kt in range(KT):
                        nc.tensor.matmul(
                            ps_q[:], wq_sb[:, kt, h * P:(h + 1) * P], xts(kt),
                            start=(kt == 0), stop=(kt == KT - 1))
                        yield
                    qt = qtsp.tile([HD, SCH], bf16, tag="qts",
                                   name=f"qt{sc}_{h}")
                    rope_evict(ps_q, qt, 0, s0)
                    qts_out.append(qt)

                def k_chain():
                    ps_k = psp.tile([P, SCH], f32, tag="p", bufs=2)
                    for kt in range(KT):
                        nc.tensor.matmul(ps_k[:], wk_sb[:, kt, :], xts(kt),
                                         start=(kt == 0), stop=(kt == KT - 1))
                        yield
                    rope_evict(ps_k, kTc[sc], 0, s0)

                def v_chain():
                    ps_v = psp.tile([P, SCH], f32, tag="p", bufs=2)
                    for kt in range(KT):
                        nc.tensor.matmul(ps_v[:], wv_sb[:, kt, :], xts(kt),
                                         start=(kt == 0), stop=(kt == KT - 1))
                        yield
                    vc = work.tile([HD, SCH], bf16, tag="vc", bufs=2)
                    nc.vector.tensor_copy(vc[:], ps_v[:])
                    for half in range(NQ):
                        if USE_XBAR_T:
                            nc.sync.dma_start_transpose(
                                out=vpc[sc][:, half, 0:HD],
                                in_=vc[:, half * P:(half + 1) * P])
                        else:
                            ps_vt = psp.tile([P, P], bf16, tag="c", bufs=2)
                            nc.tensor.transpose(
                                ps_vt[:], vc[:, half * P:(half + 1) * P],
                                identb[:])
                            nc.vector.tensor_copy(vpc[sc][:, half, 0:HD],
                                                  ps_vt[:])

                if first:
                    # chunk 0 is DMA-paced: run all six chains kt-interleaved
                    # so each arriving x slice feeds 6 matmuls (PE-bound).
                    # K/V on the "p" psum banks, Q0..Q3 on the "s" banks
                    # (idle until attention 0 starts).
                    ps_k = psp.tile([P, SCH], f32, tag="p", bufs=2)
                    ps_v = psp.tile([P, SCH], f32, tag="p", bufs=2)
                    ps_q = [psp.tile([P, SCH], f32, tag="s", bufs=4,
                                     name=f"psq0_{h}")
                            for h in range(HPC)]
                    for kt in range(KT):
                        st, sp = (kt == 0), (kt == KT - 1)
                        nc.tensor.matmul(ps_k[:], wk_sb[:, kt, :], xts(kt),
                                         start=st, stop=sp)
                        nc.tensor.matmul(ps_v[:], wv_sb[:, kt, :], xts(kt),
                                         start=st, stop=sp)
                        for h in range(HPC):
                            nc.tensor.matmul(
                                ps_q[h][:], wq_sb[:, kt, h * P:(h + 1) * P],
                                xts(kt), start=st, stop=sp)
                    rope_evict(ps_k, kTc[sc], 0, s0)
                    vc = work.tile([HD, SCH], bf16, tag="vc", bufs=2)
                    nc.vector.tensor_copy(vc[:], ps_v[:])
                    for half in range(NQ):
                        if USE_XBAR_T:
                            nc.sync.dma_start_transpose(
                                out=vpc[sc][:, half, 0:HD],
                                in_=vc[:, half * P:(half + 1) * P])
                        else:
                            ps_vt = psp.tile([P, P], bf16, tag="c", bufs=2)
                            nc.tensor.transpose(
                                ps_vt[:], vc[:, half * P:(half + 1) * P],
                                identb[:])
                            nc.vector.tensor_copy(vpc[sc][:, half, 0:HD],
                                                  ps_vt[:])
                    for h in range(HPC):
                        qt = qtsp.tile([HD, SCH], bf16, tag="qts",
                                       name=f"qt{sc}_{h}")
                        rope_evict(ps_q[h], qt, 0, s0)
                        qts_out.append(qt)
                    return

                chains = [k_chain(), q_chain(0), v_chain(), q_chain(1),
                          q_chain(2), q_chain(3)]
                for ch in chains:
                    yield from ch

            # ---------------- phase-2 emission (one PE matmul per yield)
            cc3s = [cc_outs[k][:].rearrange("(t p) s -> p t s", p=P)
                    for k in range(NSC - 1)]
            cc3j = [cc3_outs[j][:].rearrange("(g p) s -> p g s", p=P)
                    for j in range(2)]

            def p2_load(spl):
                ccts = []
                for g in range(KT // 4):
                    t = stream.tile([P, 4, SCH], bf16, tag="stream",
                                    name=f"cct{spl}_{g}")
                    if spl != NSC - 1:
                        nc.sync.dma_start(
                            out=t[:], in_=cc3s[spl][:, 4 * g:4 * g + 4, :])
                    ccts.append(t)
                if spl == NSC - 1:
                    # slice pair jj (head-slots 2jj,2jj+1) <- half-gather jj;
                    # issue jj-major so the later gather's triggers don't
                    # block the earlier slices' (queues serve waits in-order);
                    # alternate trigger engines to halve the dribble
                    for jj in range(2):
                        for g in range(KT // 4):
                            eng = nc.sync if g % 2 == 0 else nc.scalar
                            eng.dma_start(
                                out=ccts[g][:, 2 * jj:2 * jj + 2, :],
                                in_=cc3j[jj][:, 2 * g:2 * g + 2, :])
                return ccts

            def _p2_store(spl, ob, ps_o, last):
                out_sb = work.tile([P, SCH], f32, tag="osb", bufs=4)
                if last and ob == OC // P - 1:
                    # final store is the kernel tail: evict + store in
                    # quarters, triggers spread over two engines
                    for hh in range(4):
                        w = SCH // 4
                        nc.vector.tensor_copy(
                            out_sb[:, hh * w:(hh + 1) * w],
                            ps_o[:, hh * w:(hh + 1) * w])
                        eng = nc.gpsimd if hh % 2 == 0 else nc.sync
                        eng.dma_start(
                            out=outT[ob * P:(ob + 1) * P,
                                     spl * SCH + hh * w:
                                     spl * SCH + (hh + 1) * w],
                            in_=out_sb[:, hh * w:(hh + 1) * w])
                else:
                    nc.vector.tensor_copy(out_sb[:], ps_o[:])
                    nc.gpsimd.dma_start(
                        out=outT[ob * P:(ob + 1) * P,
                                 spl * SCH:(spl + 1) * SCH],
                        in_=out_sb[:])

            def p2_gen_ktmajor(spl, ccts, last=False):
                """post-loop mode: kt-outer / ob-inner on the (idle) scores
                psum banks - each ccts tile is fully consumed ~4us in, so
                the next split's loads pipeline; for the last split, kt in
                head-slot-major order so compute starts after gather j=0."""
                if spl == NSC - 1:
                    kts = [4 * g + 2 * jj + i for jj in range(2)
                           for g in range(NC) for i in range(2)]
                else:
                    kts = list(range(KT))
                ps_os = [psp.tile([P, SCH], f32, tag="s", bufs=4,
                                  name=f"pso{spl}_{ob}")
                         for ob in range(OC // P)]
                for i, kt in enumerate(kts):
                    st, sp = (i == 0), (i == KT - 1)
                    for ob in range(OC // P):
                        nc.tensor.matmul(
                            ps_os[ob][:], wo_sb[:, kt, ob * P:(ob + 1) * P],
                            ccts[kt // 4][:, kt % 4, :],
                            start=st, stop=sp)
                        yield
                for ob in range(OC // P):
                    _p2_store(spl, ob, ps_os[ob], last)

            def p2_gen(spl, ccts, last=False):
                for ob in range(OC // P):
                    ps_o = psp.tile([P, SCH], f32, tag="p", bufs=2)
                    for kt in range(KT):
                        nc.tensor.matmul(
                            ps_o[:], wo_sb[:, kt, ob * P:(ob + 1) * P],
                            ccts[kt // 4][:, kt % 4, :],
                            start=(kt == 0), stop=(kt == KT - 1))
                        yield
                    _p2_store(spl, ob, ps_o, last)

            def take(gen, n):
                if gen is None:
                    return
                for _ in range(n):
                    if next(gen, _SENT) is _SENT:
                        return

            _SENT = object()

            # ================= chunk 0 projections (bare; DMA-paced)
            qts_c = [[] for _ in range(NSC)]
            for _ in proj_gen(0, xtg0, qts_c[0], first=True):
                pass

            # ================= main loop: attention sc with interleaved filler
            for sc in range(NSC):
                s0 = sc * SCH
                # stage chunk sc+1's x stream + its projection generator
                if sc < NSC - 1:
                    xtg = []
                    for g in range(KT // 4):
                        t = stream.tile([P, 4, SCH], bf16, tag="stream",
                                        name=f"xt{sc + 1}_{g}")
                        nc.sync.dma_start(out=t[:],
                                          in_=xT[:, sc + 1, 4 * g:4 * g + 4, :])
                        xtg.append(t)
                    filler = proj_gen(sc + 1, xtg, qts_c[sc + 1])
                else:
                    filler = p2_gen(0, p2_load(0))
                if sc == 1:
                    for hh in range(2):
                        nc.sync.dma_start(
                            out=wo_sb[:, 16 * hh:16 * hh + 16, :],
                            in_=wo[:, 16 * hh:16 * hh + 16, :])

                nkb = NQ * sc + NQ
                for h in range(HPC):
                    qt = qts_c[sc][h]
                    # ---- scores + exp, filler between tiles
                    pts = []
                    for kb in range(nkb):
                        diag = kb - NQ * sc
                        c0 = max(0, diag) * P
                        ps_s = psp.tile([P, SCH], f32, tag="s", bufs=4)
                        nc.tensor.matmul(ps_s[:, c0:SCH],
                                         kTc[kb // NQ][:, (kb % NQ) * P:
                                                       (kb % NQ + 1) * P],
                                         qt[:, c0:SCH],
                                         start=True, stop=True)
                        pt = ptp.tile([P, SCH], bf16, tag="pt", name="pt")
                        nc.scalar.activation(out=pt[:, c0:SCH],
                                             in_=ps_s[:, c0:SCH],
                                             func=Exp, scale=SCALE)
                        if 0 <= diag:
                            nc.vector.tensor_mul(
                                pt[:, diag * P:(diag + 1) * P],
                                pt[:, diag * P:(diag + 1) * P], trim[:])
                        pts.append(pt)
                        take(filler, 2)

                    # ---- ctx chains, normalized + XBAR-transposed eviction
                    ctxn_h = work.tile([P, NQ, HD], bf16, tag="ctxn", bufs=2)
                    ctxT_h = work.tile([HD, NQ, P], bf16, tag="ctxT", bufs=2)
                    for qh in range(NQ):
                        iqc = NQ * sc + qh
                        ps_c = psp.tile([P, HD + 1], f32, tag="c", bufs=2)
                        for kb in range(iqc + 1):
                            nc.tensor.matmul(
                                ps_c[:], pts[kb][:, qh * P:(qh + 1) * P],
                                vpc[kb // NQ][:, kb % NQ, :],
                                start=(kb == 0), stop=(kb == iqc))
                        rden = work.tile([P, 1], f32, tag="rden", bufs=4)
                        nc.vector.reciprocal(rden[:], ps_c[:, HD:HD + 1])
                        nc.vector.tensor_scalar_mul(ctxn_h[:, qh, :],
                                                    ps_c[:, 0:HD], rden[:])
                        if USE_XBAR_T:
                            nc.sync.dma_start_transpose(
                                out=ctxT_h[:, qh, :], in_=ctxn_h[:, qh, :])
                        else:
                            ps_t = psp.tile([P, P], bf16, tag="c", bufs=2)
                            nc.tensor.transpose(ps_t[:], ctxn_h[:, qh, :],
                                                identb[:])
                            nc.vector.tensor_copy(ctxT_h[:, qh, :], ps_t[:])
                        take(filler, 3)
                    nc.sync.dma_start(
                        out=cc_ins[sc][h * HD:(h + 1) * HD, :],
                        in_=ctxT_h[:])
                    if sc == NSC - 1 and h % 2 == 1:
                        nc.gpsimd.collective_compute(
                            "AllGather",
                            mybir.AluOpType.bypass,
                            replica_groups=[list(range(NC))],
                            ins=[cc_ins[sc][(h - 1) * HD:(h + 1) * HD, :]],
                            outs=[cc3_outs[h // 2][:]],
                        )
                    take(filler, 8)

                take(filler, 10000)  # drain leftover filler for this chunk

                if sc < NSC - 1:
                    nc.gpsimd.collective_compute(
                        "AllGather",
                        mybir.AluOpType.bypass,
                        replica_groups=[list(range(NC))],
                        ins=[cc_ins[sc][:]],
                        outs=[cc_outs[sc][:]],
                    )

            # ================= phase 2 remainder: splits 1..3
            for spl in range(1, NSC):
                ccts = p2_load(spl)
                for _ in p2_gen_ktmajor(spl, ccts, last=(spl == NSC - 1)):
                    pass

    return nc


_PROGRAM_CACHE = {}


def _get_program():
    if "nc" not in _PROGRAM_CACHE:
        nc = _build_program()
        _split_multi_waits(nc, maxw=1)
        _PROGRAM_CACHE["nc"] = nc
    return _PROGRAM_CACHE["nc"]


def _rope_tables_T():
    inv_freq = (1.0 / (10000.0 ** (np.arange(0, HD, 2, dtype=np.float32) / HD))
                ).astype(np.float32)
    ang = np.arange(S, dtype=np.float32)[:, None] * inv_freq[None, :]  # [S,64]
    cos_h = np.cos(ang).T  # [64, S]
    sin_h = np.sin(ang).T
    cosT = np.concatenate([cos_h, cos_h], axis=0)       # [128, S]
    sinNT = np.concatenate([-sin_h, sin_h], axis=0)     # rot sign folded in
    bf = ml_dtypes.bfloat16
    return (np.ascontiguousarray(cosT).astype(bf),
            np.ascontiguousarray(sinNT).astype(bf))


def _prep_in_maps(x, Wq, Wk, Wv, Wo):
    bf = ml_dtypes.bfloat16
    x2d = np.asarray(x, np.float32).reshape(S, D)
    # chunk-major: [P, NSC, KT, SCH] so each chunk DMA reads 4KB runs
    xT_dev = np.ascontiguousarray(
        x2d.T.reshape(KT, P, NSC, SCH).transpose(1, 2, 0, 3)).astype(bf)
    cosT, sinNT = _rope_tables_T()
    tri_np = (np.arange(P)[:, None] <= np.arange(P)[None, :]).astype(bf)

    def wtiles(Wslice, width):
        return np.ascontiguousarray(
            np.asarray(Wslice, np.float32).reshape(KT, P, width)
            .transpose(1, 0, 2)).astype(bf)

    in_maps = []
    for c in range(NC):
        in_maps.append({
            "xT": xT_dev,
            "wq": wtiles(Wq[:, c * OC:(c + 1) * OC], OC),
            "wk": wtiles(Wk[:, c * HD:(c + 1) * HD], HD),
            "wv": wtiles(Wv[:, c * HD:(c + 1) * HD], HD),
            "wo": wtiles(Wo[:, c * OC:(c + 1) * OC], OC),
            "cosT": cosT,
            "sinNT": sinNT,
            "tri": tri_np,
        })
    return in_maps


def _run(inputs, trace=False):
    nc = _get_program()
    in_maps = _prep_in_maps(inputs["x"], inputs["Wq"], inputs["Wk"],
                            inputs["Wv"], inputs["Wo"])
    res = run_bass_kernel_spmd(nc, in_maps, core_ids=list(range(NC)),
                               trace=trace)
    out = np.empty((S, D), np.float32)
    for c in range(NC):
        out[:, c * OC:(c + 1) * OC] = res.results[c]["outT"].T
    return out.reshape(B, S, D), res


def kernel(**inputs):
    out, _ = _run(inputs, trace=False)
    return out
